# revision 36
# baseline (speedup 1.0000x reference)
"""NoPropCT MomentNet kernel for Trainium2 (Bass/Tile), 8-core data parallel.

Reference computation: 10 Euler steps of
    state <- state + dt * MLP(concat([state, eta, t]))
with MLP 17->64->64->32->8 (swish), state_0 = eta, dt = 0.1.

Exact restructuring (same math as the reference, not approximate):
  pre1_k := W1s^T state_k + W1e^T eta          (layer-1 preact minus bias)
  pre1_{k+1} = pre1_k + (dt*W4@W1s)^T h3_k     (G1 update)
  out        = eta + (dt*W4)^T (sum_k h3_k) + b4
  bias1_k    = b1 + k*dt*Wt1 + k*dt*(b4@W1s)
so the state is never materialized and the W4 output matmul happens once
per group on the step-summed h3 instead of every step.

Mapping:
- eta is DMA'd in natural layout (contiguous 2KB/partition lines) and
  transposed on the tensor engine (identity matmul); same on the way out.
  This avoids 4-byte-granular DMA gathers entirely.
- Activations (the throughput wall: scalar engine is 1 elem/cycle/lane)
  are emitted as few, large instructions; matmul operands are bf16
  (1 cycle/row on the PE vs 4 for fp32); accumulators stay fp32.
- Batch is processed in groups of 8192 rows = 16 slots x 512 cols;
  two groups are software-pipelined (interleaved emission, separate PSUM
  pools) so the scalar engine never idles in the du->pre1 dependency tail.
"""

import numpy as np

import concourse.bass as bass
import concourse.tile as tile
from concourse import bacc, mybir
from concourse.bass_utils import run_bass_kernel_spmd

ETA_DIM = 8
NUM_STEPS = 10
DT = np.float32(1.0 / NUM_STEPS)
BATCH = 2097152
N_CORES = 8
BC = BATCH // N_CORES   # per-core batch
GROUP = 8192            # rows per compute group (16 slots x 512 cols)
FP32 = mybir.dt.float32
BF16 = mybir.dt.bfloat16
BF16_NP = mybir.dt.np(mybir.dt.bfloat16)

# bf16 weight blob columns [128, W_COLS]
C_W2 = 0      # [128,128] blkdiag(W2, W2)
C_W3 = 128    # [128,64]  blkdiag(W3, W3)
C_G1 = 192    # [128,128] blkdiag(G1, G1), duplicated on both 64-halves
C_GO = 320    # [128,32]  4-way blkdiag of GO = dt*W4
C_I1E = 352   # [128,128] layer1-init lhsT for even slot-pairs
C_I1O = 480   # [128,128] layer1-init lhsT for odd slot-pairs
W_COLS = 608

# fp32 blob columns [128, WF_COLS]
C_ID = 0      # [128,128] identity (PE transpose)
C_B1 = 128    # 10 cols: per-step swish1 bias (dup x2 on 64-halves)
C_B2 = 138    # b2 dup x2
C_B3 = 139    # b3 tiled x4
C_B4 = 140    # b4 tiled x16
C_GOF = 141   # [128,32] fp32 copy of the GO block (h3sum matmul is fp32)
WF_COLS = 173


def build_host_params(W1, b1, W2, b2, W3, b3, W4, b4):
    W1s, W1e, Wt1 = W1[0:8], W1[8:16], W1[16]
    A1 = (W1s + W1e).astype(np.float32)          # [8,64]
    G1 = (DT * (W4 @ W1s)).astype(np.float32)    # [32,64]
    GO = (DT * W4).astype(np.float32)            # [32,8]

    wb = np.zeros((128, W_COLS), np.float32)
    wb[0:64, C_W2:C_W2 + 64] = W2
    wb[64:128, C_W2 + 64:C_W2 + 128] = W2
    wb[0:64, C_W3:C_W3 + 32] = W3
    wb[64:128, C_W3 + 32:C_W3 + 64] = W3
    for b in (0, 64):
        wb[b:b + 32, C_G1:C_G1 + 64] = G1
        wb[b + 32:b + 64, C_G1 + 64:C_G1 + 128] = G1
    for c in range(4):
        wb[32 * c:32 * c + 32, C_GO + 8 * c:C_GO + 8 * c + 8] = GO
    for q in range(4):
        r = 32 * q
        wb[r:r + 8, C_I1E:C_I1E + 64] = A1
        wb[r + 8:r + 16, C_I1E + 64:C_I1E + 128] = A1
        wb[r + 16:r + 24, C_I1O:C_I1O + 64] = A1
        wb[r + 24:r + 32, C_I1O + 64:C_I1O + 128] = A1

    wf = np.zeros((128, WF_COLS), np.float32)
    wf[:, C_ID:C_ID + 128] = np.eye(128, dtype=np.float32)
    b4W1s = (b4 @ W1s).astype(np.float32)        # [64]
    for k in range(NUM_STEPS):
        bias1 = b1 + (k * DT) * Wt1 + (k * DT) * b4W1s
        wf[0:64, C_B1 + k] = bias1
        wf[64:128, C_B1 + k] = bias1
    wf[0:64, C_B2] = b2
    wf[64:128, C_B2] = b2
    for a in range(4):
        wf[32 * a:32 * a + 32, C_B3] = b3
    wf[:, C_B4] = np.tile(b4, 16)
    wf[:, C_GOF:C_GOF + 32] = wb[:, C_GO:C_GO + 32]
    return wb.astype(BF16_NP), wf


class _GroupState:
    """SBUF tiles carried across a group's lifetime."""
    __slots__ = ("stage", "etaT", "pre1", "h1", "h2", "h3", "h3sum", "base")


def build_nc(bc=BC):
    assert bc % (2 * GROUP) == 0
    n_pairs = bc // (2 * GROUP)
    silu = mybir.ActivationFunctionType.Silu
    ADD = mybir.AluOpType.add

    nc = bacc.Bacc("TRN2", target_bir_lowering=False, debug=False)
    eta_d = nc.declare_dram_parameter("eta", [bc, ETA_DIM], FP32, isOutput=False)
    wb_d = nc.declare_dram_parameter("wb", [128, W_COLS], BF16, isOutput=False)
    wf_d = nc.declare_dram_parameter("wf", [128, WF_COLS], FP32, isOutput=False)
    out_d = nc.declare_dram_parameter("out", [bc, ETA_DIM], FP32, isOutput=True)

    mm = nc.tensor.matmul

    with tile.TileContext(nc) as tc:
        with (
            tc.tile_pool(name="wpool", bufs=1) as wpool,
            tc.tile_pool(name="spool", bufs=4) as spool,
            tc.tile_pool(name="opool", bufs=4) as opool,
            tc.tile_pool(name="epool", bufs=3) as epool,
            tc.tile_pool(name="prepool", bufs=3) as prepool,
            tc.tile_pool(name="h1pool", bufs=3) as h1pool,
            tc.tile_pool(name="h2pool", bufs=3) as h2pool,
            tc.tile_pool(name="h3pool", bufs=3) as h3pool,
            tc.tile_pool(name="hspool", bufs=3) as hspool,
            tc.tile_pool(name="otpool", bufs=2) as otpool,
            tc.tile_pool(name="psA", bufs=2, space=bass.MemorySpace.PSUM) as psA,
            tc.tile_pool(name="psB", bufs=2, space=bass.MemorySpace.PSUM) as psB,
        ):
            wb = wpool.tile([128, W_COLS], BF16)
            wf = wpool.tile([128, WF_COLS], FP32)
            nc.sync.dma_start(wb[:], wb_d[:])
            nc.sync.dma_start(wf[:], wf_d[:])
            ident = wf[:, C_ID:C_ID + 128]

            def bias(c):
                return wf[:, c:c + 1]

            def load(g):
                """Prefetch group g's eta slice (natural layout, 2KB/line)."""
                st = _GroupState()
                st.base = g * GROUP
                st.stage = spool.tile([128, 512], FP32, tag="stage")
                src = eta_d[st.base:st.base + GROUP, :]
                nc.sync.dma_start(
                    st.stage[:], src.rearrange("(p r) f -> p (r f)", p=128))
                return st

            def prep_t(st, ps):
                """PE transpose of a loaded group into etaT."""
                psT = ps.tile([128, 1024], FP32, tag="ps")
                for t in range(4):
                    mm(psT[:, 128 * t:128 * t + 128],
                       st.stage[:, 128 * t:128 * t + 128], ident,
                       is_transpose=True, start=True, stop=True)
                st.etaT = epool.tile([128, 512], BF16, tag="etaT")
                nc.vector.tensor_copy(st.etaT[:], psT[:, 0:512])

            def prep_i(st, ps):
                """Layer-1 init: pre1_0 = A1^T etaT, built in 4 psum chunks."""
                st.pre1 = prepool.tile([128, 4096], FP32, tag="pre1")
                for v in range(4):  # chunk v = pairs 2v, 2v+1
                    pi = ps.tile([128, 1024], FP32, tag="ps")
                    for i in range(2):
                        s = 2 * v + i
                        q, odd = divmod(s, 2)
                        col = C_I1O if odd else C_I1E
                        mm(pi[:, 512 * i:512 * i + 512],
                           wb[32 * q:32 * q + 32, col:col + 128],
                           st.etaT[32 * q:32 * q + 32, :],
                           start=True, stop=True, tile_position=(32 * q, 0))
                    nc.vector.tensor_copy(
                        st.pre1[:, 1024 * v:1024 * v + 1024], pi[:])
                st.h1 = h1pool.tile([128, 4096], BF16, tag="h1")
                st.h2 = h2pool.tile([128, 4096], BF16, tag="h2")
                st.h3 = h3pool.tile([128, 2048], BF16, tag="h3")
                st.h3sum = hspool.tile([128, 2048], FP32, tag="h3sum")
                return st

            def step_h1(st, k):
                # h1 in halves: half h depends only on pre1 adds 2h, 2h+1 of
                # the previous step, shortening the cross-step critical path
                for h in range(2):
                    nc.scalar.activation(
                        st.h1[:, 2048 * h:2048 * h + 2048],
                        st.pre1[:, 2048 * h:2048 * h + 2048], silu,
                        bias=bias(C_B1 + k))

            def step(st, k, ps, skip_h1=False):
                last = k == NUM_STEPS - 1
                if not skip_h1:
                    step_h1(st, k)
                # layer 2: 4 psum chunks of [128,1024] = 2 slot-pairs each
                for c in range(4):
                    p2 = ps.tile([128, 1024], FP32, tag="ps")
                    for i in range(2):
                        s = 2 * c + i
                        mm(p2[:, 512 * i:512 * i + 512],
                           wb[:, C_W2:C_W2 + 128],
                           st.h1[:, 512 * s:512 * s + 512],
                           start=True, stop=True)
                    nc.scalar.activation(
                        st.h2[:, 1024 * c:1024 * c + 1024], p2[:], silu,
                        bias=bias(C_B2))
                # layer 3: 2 psum chunks [128,1024]; u-block u = pairs 2u,2u+1
                for c in range(2):
                    p3 = ps.tile([128, 1024], FP32, tag="ps")
                    for du_ in range(2):
                        u = 2 * c + du_
                        for b in range(2):
                            s = 2 * u + b
                            mm(p3[64 * b:64 * b + 64,
                                  512 * du_:512 * du_ + 512],
                               wb[:, C_W3:C_W3 + 64],
                               st.h2[:, 512 * s:512 * s + 512],
                               start=True, stop=True, tile_position=(0, 64 * b))
                    nc.scalar.activation(
                        st.h3[:, 1024 * c:1024 * c + 1024], p3[:], silu,
                        bias=bias(C_B3))
                # h3sum: middle steps on the otherwise-idle gpsimd engine;
                # first/last on DVE (gpsimd is ~3x slower per op, and the
                # last add gates the tail's GO matmul)
                if k == 0:
                    nc.vector.tensor_copy(st.h3sum[:], st.h3[:])
                elif last:
                    nc.vector.tensor_tensor(st.h3sum[:], st.h3sum[:],
                                            st.h3[:], ADD)
                else:
                    nc.gpsimd.tensor_tensor(st.h3sum[:], st.h3sum[:],
                                            st.h3[:], ADD)
                if last:
                    return  # pre1 is dead after the final step
                # G1 update: 4 du chunks [128,1024] = pairs 2d, 2d+1
                for d in range(4):
                    du = ps.tile([128, 1024], FP32, tag="ps")
                    for j in range(2):
                        s = 2 * d + j
                        u, b = s // 2, s % 2
                        mm(du[:, 512 * j:512 * j + 512],
                           wb[64 * b:64 * b + 64, C_G1:C_G1 + 128],
                           st.h3[64 * b:64 * b + 64, 512 * u:512 * u + 512],
                           start=True, stop=True, tile_position=(64 * b, 0))
                    nc.vector.tensor_tensor(
                        st.pre1[:, 1024 * d:1024 * d + 1024],
                        st.pre1[:, 1024 * d:1024 * d + 1024], du[:], ADD)

            def tail(st, ps):
                """GO matmul on h3sum, add b4, transpose back, add eta, DMA."""
                po = ps.tile([128, 1024], FP32, tag="ps")
                outp = po[:, 0:512]
                for u in range(4):
                    mm(outp[32 * u:32 * u + 32, :],
                       wf[:, C_GOF:C_GOF + 32],
                       st.h3sum[:, 512 * u:512 * u + 512],
                       start=True, stop=True, tile_position=(0, 32 * u))
                outT = otpool.tile([128, 512], FP32, tag="outT")
                nc.vector.tensor_scalar_add(outT[:], outp, bias(C_B4))
                po2 = ps.tile([128, 1024], FP32, tag="ps")
                for t in range(4):
                    mm(po2[:, 128 * t:128 * t + 128],
                       outT[:, 128 * t:128 * t + 128], ident,
                       is_transpose=True, start=True, stop=True)
                outst = opool.tile([128, 512], FP32, tag="outst")
                nc.vector.tensor_tensor(outst[:], po2[:, 0:512],
                                        st.stage[:], ADD)
                dst = out_d[st.base:st.base + GROUP, :]
                nc.sync.dma_start(
                    dst.rearrange("(p r) f -> p (r f)", p=128), outst[:])

            # Software pipeline: loads run 1 pair ahead (SP queue ordering
            # puts them before this pair's output DMAs); preps for pair i+1
            # are emitted before pair i's tails so the PE/DVE fronts overlap
            # with pair i's last steps instead of serializing at the boundary.
            stA, stB = load(0), load(1)
            prep_t(stA, psA)
            prep_i(stA, psA)
            prep_t(stB, psB)
            prep_i(stB, psB)
            # Tails are deferred past the NEXT pair's first step so the
            # in-order PE queue services the new pair's first matmul wave
            # before the old pair's GO/transpose-out chain.
            pend = None
            for i in range(n_pairs):
                for k in range(NUM_STEPS):
                    if k == 0:
                        # interleave both streams' first h1 so ACT has
                        # stream-B work during stream-A's first PE wave
                        step_h1(stA, 0)
                        step_h1(stB, 0)
                        step(stA, 0, psA, skip_h1=True)
                        step(stB, 0, psB, skip_h1=True)
                    else:
                        step(stA, k, psA)
                        step(stB, k, psB)
                    # split the two deferred tails across step windows so
                    # their DVE/PE work does not pile into one step's slack
                    if k == 0 and pend is not None:
                        tail(pend[0], psA)
                    if k == 1 and pend is not None:
                        tail(pend[1], psB)
                        pend = None
                    # prep next pair while late steps run, so its first h1
                    # is not gated on the boundary init chain; spread the
                    # transpose and init sub-phases over four step windows
                    # so no single step's pre1 adds sit behind the full
                    # init-copy DVE injection
                    if i + 1 < n_pairs:
                        if k == 5:
                            nA, nB = load(2 * i + 2), load(2 * i + 3)
                            prep_t(nA, psA)
                        elif k == 6:
                            prep_i(nA, psA)
                        elif k == 7:
                            prep_t(nB, psB)
                        elif k == 8:
                            prep_i(nB, psB)
                pend = (stA, stB)
                if i + 1 < n_pairs:
                    stA, stB = nA, nB
            tail(pend[0], psA)
            tail(pend[1], psB)
    nc.compile()
    return nc


_NC_CACHE = {}


def kernel(eta, W1, b1, W2, b2, W3, b3, W4, b4):
    eta = np.asarray(eta, np.float32)
    wb, wf = build_host_params(
        np.asarray(W1, np.float32), np.asarray(b1, np.float32),
        np.asarray(W2, np.float32), np.asarray(b2, np.float32),
        np.asarray(W3, np.float32), np.asarray(b3, np.float32),
        np.asarray(W4, np.float32), np.asarray(b4, np.float32))
    if BC not in _NC_CACHE:
        _NC_CACHE[BC] = build_nc(BC)
    nc = _NC_CACHE[BC]
    core_ids = list(range(N_CORES))
    in_maps = [{"eta": np.ascontiguousarray(eta[i * BC:(i + 1) * BC]),
                "wb": wb, "wf": wf} for i in core_ids]
    res = run_bass_kernel_spmd(nc, in_maps, core_ids)
    out = np.concatenate([res.results[i]["out"] for i in core_ids], axis=0)
    return out.astype(np.float32)


# revision 37
# speedup vs baseline: 1.0217x; 1.0217x over previous
"""NoPropCT MomentNet kernel for Trainium2 (Bass/Tile), 8-core data parallel.

Reference computation: 10 Euler steps of
    state <- state + dt * MLP(concat([state, eta, t]))
with MLP 17->64->64->32->8 (swish), state_0 = eta, dt = 0.1.

Exact restructuring (same math as the reference, not approximate):
  pre1_k := W1s^T state_k + W1e^T eta          (layer-1 preact minus bias)
  pre1_{k+1} = pre1_k + (dt*W4@W1s)^T h3_k     (G1 update)
  out        = eta + (dt*W4)^T (sum_k h3_k) + b4
  bias1_k    = b1 + k*dt*Wt1 + k*dt*(b4@W1s)
so the state is never materialized and the W4 output matmul happens once
per group on the step-summed h3 instead of every step.

Mapping:
- eta is DMA'd in natural layout (contiguous 2KB/partition lines) and
  transposed on the tensor engine (identity matmul); same on the way out.
  This avoids 4-byte-granular DMA gathers entirely.
- Activations (the throughput wall: scalar engine is 1 elem/cycle/lane)
  are emitted as few, large instructions; matmul operands are bf16
  (1 cycle/row on the PE vs 4 for fp32); accumulators stay fp32.
- Batch is processed in groups of 8192 rows = 16 slots x 512 cols;
  two groups are software-pipelined (interleaved emission, separate PSUM
  pools) so the scalar engine never idles in the du->pre1 dependency tail.
"""

import numpy as np

import concourse.bass as bass
import concourse.tile as tile
from concourse import bacc, mybir
from concourse.bass_utils import run_bass_kernel_spmd

ETA_DIM = 8
NUM_STEPS = 10
DT = np.float32(1.0 / NUM_STEPS)
BATCH = 2097152
N_CORES = 8
BC = BATCH // N_CORES   # per-core batch
GROUP = 8192            # rows per compute group (16 slots x 512 cols)
FP32 = mybir.dt.float32
BF16 = mybir.dt.bfloat16
BF16_NP = mybir.dt.np(mybir.dt.bfloat16)

# bf16 weight blob columns [128, W_COLS]
C_W2 = 0      # [128,128] blkdiag(W2, W2)
C_W3 = 128    # [128,64]  blkdiag(W3, W3)
C_G1 = 192    # [128,128] blkdiag(G1, G1), duplicated on both 64-halves
C_GO = 320    # [128,32]  4-way blkdiag of GO = dt*W4
C_I1E = 352   # [128,128] layer1-init lhsT for even slot-pairs
C_I1O = 480   # [128,128] layer1-init lhsT for odd slot-pairs
W_COLS = 608

# fp32 blob columns [128, WF_COLS]
C_ID = 0      # [128,128] identity (PE transpose)
C_B1 = 128    # 10 cols: per-step swish1 bias (dup x2 on 64-halves)
C_B2 = 138    # b2 dup x2
C_B3 = 139    # b3 tiled x4
C_B4 = 140    # b4 tiled x16
C_GOF = 141   # [128,32] fp32 copy of the GO block (h3sum matmul is fp32)
WF_COLS = 173


def build_host_params(W1, b1, W2, b2, W3, b3, W4, b4):
    W1s, W1e, Wt1 = W1[0:8], W1[8:16], W1[16]
    A1 = (W1s + W1e).astype(np.float32)          # [8,64]
    G1 = (DT * (W4 @ W1s)).astype(np.float32)    # [32,64]
    GO = (DT * W4).astype(np.float32)            # [32,8]

    wb = np.zeros((128, W_COLS), np.float32)
    wb[0:64, C_W2:C_W2 + 64] = W2
    wb[64:128, C_W2 + 64:C_W2 + 128] = W2
    wb[0:64, C_W3:C_W3 + 32] = W3
    wb[64:128, C_W3 + 32:C_W3 + 64] = W3
    for b in (0, 64):
        wb[b:b + 32, C_G1:C_G1 + 64] = G1
        wb[b + 32:b + 64, C_G1 + 64:C_G1 + 128] = G1
    for c in range(4):
        wb[32 * c:32 * c + 32, C_GO + 8 * c:C_GO + 8 * c + 8] = GO
    for q in range(4):
        r = 32 * q
        wb[r:r + 8, C_I1E:C_I1E + 64] = A1
        wb[r + 8:r + 16, C_I1E + 64:C_I1E + 128] = A1
        wb[r + 16:r + 24, C_I1O:C_I1O + 64] = A1
        wb[r + 24:r + 32, C_I1O + 64:C_I1O + 128] = A1

    wf = np.zeros((128, WF_COLS), np.float32)
    wf[:, C_ID:C_ID + 128] = np.eye(128, dtype=np.float32)
    b4W1s = (b4 @ W1s).astype(np.float32)        # [64]
    for k in range(NUM_STEPS):
        bias1 = b1 + (k * DT) * Wt1 + (k * DT) * b4W1s
        wf[0:64, C_B1 + k] = bias1
        wf[64:128, C_B1 + k] = bias1
    wf[0:64, C_B2] = b2
    wf[64:128, C_B2] = b2
    for a in range(4):
        wf[32 * a:32 * a + 32, C_B3] = b3
    wf[:, C_B4] = np.tile(b4, 16)
    wf[:, C_GOF:C_GOF + 32] = wb[:, C_GO:C_GO + 32]
    return wb.astype(BF16_NP), wf


class _GroupState:
    """SBUF tiles carried across a group's lifetime."""
    __slots__ = ("stage", "etaT", "pre1", "h1", "h2", "h3", "h3sum", "base")


def build_nc(bc=BC):
    assert bc % (2 * GROUP) == 0
    n_pairs = bc // (2 * GROUP)
    silu = mybir.ActivationFunctionType.Silu
    ADD = mybir.AluOpType.add

    nc = bacc.Bacc("TRN2", target_bir_lowering=False, debug=False)
    eta_d = nc.declare_dram_parameter("eta", [bc, ETA_DIM], FP32, isOutput=False)
    wb_d = nc.declare_dram_parameter("wb", [128, W_COLS], BF16, isOutput=False)
    wf_d = nc.declare_dram_parameter("wf", [128, WF_COLS], FP32, isOutput=False)
    out_d = nc.declare_dram_parameter("out", [bc, ETA_DIM], FP32, isOutput=True)

    mm = nc.tensor.matmul

    with tile.TileContext(nc) as tc:
        with (
            tc.tile_pool(name="wpool", bufs=1) as wpool,
            tc.tile_pool(name="spool", bufs=4) as spool,
            tc.tile_pool(name="opool", bufs=4) as opool,
            tc.tile_pool(name="epool", bufs=3) as epool,
            tc.tile_pool(name="prepool", bufs=3) as prepool,
            tc.tile_pool(name="h1pool", bufs=3) as h1pool,
            tc.tile_pool(name="h2pool", bufs=3) as h2pool,
            tc.tile_pool(name="h3pool", bufs=3) as h3pool,
            tc.tile_pool(name="hspool", bufs=3) as hspool,
            tc.tile_pool(name="otpool", bufs=2) as otpool,
            tc.tile_pool(name="psA", bufs=2, space=bass.MemorySpace.PSUM) as psA,
            tc.tile_pool(name="psB", bufs=2, space=bass.MemorySpace.PSUM) as psB,
        ):
            wb = wpool.tile([128, W_COLS], BF16)
            wf = wpool.tile([128, WF_COLS], FP32)
            nc.sync.dma_start(wb[:], wb_d[:])
            nc.sync.dma_start(wf[:], wf_d[:])
            ident = wf[:, C_ID:C_ID + 128]

            def bias(c):
                return wf[:, c:c + 1]

            def load(g):
                """Prefetch group g's eta slice (natural layout, 2KB/line)."""
                st = _GroupState()
                st.base = g * GROUP
                st.stage = spool.tile([128, 512], FP32, tag="stage")
                src = eta_d[st.base:st.base + GROUP, :]
                nc.sync.dma_start(
                    st.stage[:], src.rearrange("(p r) f -> p (r f)", p=128))
                return st

            def prep(st, ps):
                """PE transpose + layer-1 init for a loaded group."""
                psT = ps.tile([128, 1024], FP32, tag="ps")
                for t in range(4):
                    mm(psT[:, 128 * t:128 * t + 128],
                       st.stage[:, 128 * t:128 * t + 128], ident,
                       is_transpose=True, start=True, stop=True)
                st.etaT = epool.tile([128, 512], BF16, tag="etaT")
                nc.vector.tensor_copy(st.etaT[:], psT[:, 0:512])

                # layer-1 init: pre1_0 = A1^T etaT, built in 4 psum chunks
                st.pre1 = prepool.tile([128, 4096], FP32, tag="pre1")
                for v in range(4):  # chunk v = pairs 2v, 2v+1
                    pi = ps.tile([128, 1024], FP32, tag="ps")
                    for i in range(2):
                        s = 2 * v + i
                        q, odd = divmod(s, 2)
                        col = C_I1O if odd else C_I1E
                        mm(pi[:, 512 * i:512 * i + 512],
                           wb[32 * q:32 * q + 32, col:col + 128],
                           st.etaT[32 * q:32 * q + 32, :],
                           start=True, stop=True, tile_position=(32 * q, 0))
                    nc.vector.tensor_copy(
                        st.pre1[:, 1024 * v:1024 * v + 1024], pi[:])
                st.h1 = h1pool.tile([128, 4096], BF16, tag="h1")
                st.h2 = h2pool.tile([128, 4096], BF16, tag="h2")
                st.h3 = h3pool.tile([128, 2048], BF16, tag="h3")
                st.h3sum = hspool.tile([128, 2048], FP32, tag="h3sum")
                return st

            def step_h1(st, k):
                # h1 in halves: half h depends only on pre1 adds 2h, 2h+1 of
                # the previous step, shortening the cross-step critical path
                for h in range(2):
                    nc.scalar.activation(
                        st.h1[:, 2048 * h:2048 * h + 2048],
                        st.pre1[:, 2048 * h:2048 * h + 2048], silu,
                        bias=bias(C_B1 + k))

            def step(st, k, ps, skip_h1=False):
                last = k == NUM_STEPS - 1
                if not skip_h1:
                    step_h1(st, k)
                # layer 2: 4 psum chunks of [128,1024] = 2 slot-pairs each
                for c in range(4):
                    p2 = ps.tile([128, 1024], FP32, tag="ps")
                    for i in range(2):
                        s = 2 * c + i
                        mm(p2[:, 512 * i:512 * i + 512],
                           wb[:, C_W2:C_W2 + 128],
                           st.h1[:, 512 * s:512 * s + 512],
                           start=True, stop=True)
                    nc.scalar.activation(
                        st.h2[:, 1024 * c:1024 * c + 1024], p2[:], silu,
                        bias=bias(C_B2))
                # layer 3: 2 psum chunks [128,1024]; u-block u = pairs 2u,2u+1
                for c in range(2):
                    p3 = ps.tile([128, 1024], FP32, tag="ps")
                    for du_ in range(2):
                        u = 2 * c + du_
                        for b in range(2):
                            s = 2 * u + b
                            mm(p3[64 * b:64 * b + 64,
                                  512 * du_:512 * du_ + 512],
                               wb[:, C_W3:C_W3 + 64],
                               st.h2[:, 512 * s:512 * s + 512],
                               start=True, stop=True, tile_position=(0, 64 * b))
                    nc.scalar.activation(
                        st.h3[:, 1024 * c:1024 * c + 1024], p3[:], silu,
                        bias=bias(C_B3))
                # h3sum: middle steps on the otherwise-idle gpsimd engine;
                # first/last on DVE (gpsimd is ~3x slower per op, and the
                # last add gates the tail's GO matmul)
                if k == 0:
                    nc.vector.tensor_copy(st.h3sum[:], st.h3[:])
                elif last:
                    nc.vector.tensor_tensor(st.h3sum[:], st.h3sum[:],
                                            st.h3[:], ADD)
                else:
                    nc.gpsimd.tensor_tensor(st.h3sum[:], st.h3sum[:],
                                            st.h3[:], ADD)
                if last:
                    return  # pre1 is dead after the final step
                # G1 update: 4 du chunks [128,1024] = pairs 2d, 2d+1
                for d in range(4):
                    du = ps.tile([128, 1024], FP32, tag="ps")
                    for j in range(2):
                        s = 2 * d + j
                        u, b = s // 2, s % 2
                        mm(du[:, 512 * j:512 * j + 512],
                           wb[64 * b:64 * b + 64, C_G1:C_G1 + 128],
                           st.h3[64 * b:64 * b + 64, 512 * u:512 * u + 512],
                           start=True, stop=True, tile_position=(64 * b, 0))
                    nc.vector.tensor_tensor(
                        st.pre1[:, 1024 * d:1024 * d + 1024],
                        st.pre1[:, 1024 * d:1024 * d + 1024], du[:], ADD)

            def tail(st, ps):
                """GO matmul on h3sum, add b4, transpose back, add eta, DMA."""
                po = ps.tile([128, 1024], FP32, tag="ps")
                outp = po[:, 0:512]
                for u in range(4):
                    mm(outp[32 * u:32 * u + 32, :],
                       wf[:, C_GOF:C_GOF + 32],
                       st.h3sum[:, 512 * u:512 * u + 512],
                       start=True, stop=True, tile_position=(0, 32 * u))
                outT = otpool.tile([128, 512], FP32, tag="outT")
                nc.vector.tensor_scalar_add(outT[:], outp, bias(C_B4))
                po2 = ps.tile([128, 1024], FP32, tag="ps")
                for t in range(4):
                    mm(po2[:, 128 * t:128 * t + 128],
                       outT[:, 128 * t:128 * t + 128], ident,
                       is_transpose=True, start=True, stop=True)
                outst = opool.tile([128, 512], FP32, tag="outst")
                nc.vector.tensor_tensor(outst[:], po2[:, 0:512],
                                        st.stage[:], ADD)
                dst = out_d[st.base:st.base + GROUP, :]
                nc.sync.dma_start(
                    dst.rearrange("(p r) f -> p (r f)", p=128), outst[:])

            # Software pipeline: loads run 1 pair ahead (SP queue ordering
            # puts them before this pair's output DMAs); preps for pair i+1
            # are emitted before pair i's tails so the PE/DVE fronts overlap
            # with pair i's last steps instead of serializing at the boundary.
            stA, stB = load(0), load(1)
            prep(stA, psA)
            prep(stB, psB)
            # Tails are deferred past the NEXT pair's first step so the
            # in-order PE queue services the new pair's first matmul wave
            # before the old pair's GO/transpose-out chain.
            pend = None
            for i in range(n_pairs):
                for k in range(NUM_STEPS):
                    if k == 0:
                        # interleave both streams' first h1 so ACT has
                        # stream-B work during stream-A's first PE wave
                        step_h1(stA, 0)
                        step_h1(stB, 0)
                        step(stA, 0, psA, skip_h1=True)
                        step(stB, 0, psB, skip_h1=True)
                    else:
                        step(stA, k, psA)
                        step(stB, k, psB)
                    # split the two deferred tails across step windows so
                    # their DVE/PE work does not pile into one step's slack
                    if k == 0 and pend is not None:
                        tail(pend[0], psA)
                    if k == 1 and pend is not None:
                        tail(pend[1], psB)
                        pend = None
                    # prep next pair while late steps run, so its first h1
                    # is not gated on the boundary init chain; split across
                    # two step windows so the init-copy DVE work does not
                    # delay one step's pre1 adds by the full ~12us
                    if k == 6 and i + 1 < n_pairs:
                        nA, nB = load(2 * i + 2), load(2 * i + 3)
                        prep(nA, psA)
                    if k == 8 and i + 1 < n_pairs:
                        prep(nB, psB)
                pend = (stA, stB)
                if i + 1 < n_pairs:
                    stA, stB = nA, nB
            tail(pend[0], psA)
            tail(pend[1], psB)
    nc.compile()
    return nc


_NC_CACHE = {}


def kernel(eta, W1, b1, W2, b2, W3, b3, W4, b4):
    eta = np.asarray(eta, np.float32)
    wb, wf = build_host_params(
        np.asarray(W1, np.float32), np.asarray(b1, np.float32),
        np.asarray(W2, np.float32), np.asarray(b2, np.float32),
        np.asarray(W3, np.float32), np.asarray(b3, np.float32),
        np.asarray(W4, np.float32), np.asarray(b4, np.float32))
    if BC not in _NC_CACHE:
        _NC_CACHE[BC] = build_nc(BC)
    nc = _NC_CACHE[BC]
    core_ids = list(range(N_CORES))
    in_maps = [{"eta": np.ascontiguousarray(eta[i * BC:(i + 1) * BC]),
                "wb": wb, "wf": wf} for i in core_ids]
    res = run_bass_kernel_spmd(nc, in_maps, core_ids)
    out = np.concatenate([res.results[i]["out"] for i in core_ids], axis=0)
    return out.astype(np.float32)
